# revision 23
# baseline (speedup 1.0000x reference)
"""Trainium2 Bass kernel for nn_MultiHeadAttention (B=4, S=2048, D=1024, H=16).

Sharding: 8 cores = batch(4) x head-half(2).  Each core computes, for its
batch element, 8 of the 16 heads: QKV projections against column-sliced
weights, causal attention, and the output projection against the matching
row-slice of Wo.  The two partial outputs per batch element are summed on
the host (replaces the tensor-parallel all-reduce), and Wo_b is added there.

v3: single fused pipeline.  The QKV projections of the next head-pair and
the output projection are interleaved into the attention kb-loop as PE
"filler" units, so the tensor engine keeps streaming while the scalar
engine runs exp (an idle PE lets the HAM clock gate throttle it to
1.2 GHz).  All matmuls use the uniform (128,128) PE tiling mode (q is
stored zero-padded per head).  Softmax normalization avoids GpSimd
entirely: one DVE copy frees the PSUM accumulator, a stride-0-source DMA
broadcasts the denominator row across 64 partitions, and DVE does
reciprocal + multiply.  Weights arrive pair-major so each pair's slice is
one contiguous DMA, with the x tiles split across both HW DMA rings.
"""

import sys

if "/opt/trn_rl_repo" not in sys.path:
    sys.path.insert(0, "/opt/trn_rl_repo")

import numpy as np
import ml_dtypes

B, S, D = 4, 2048, 1024
H, HD = 16, 64
HH = H // 2          # heads per core
DH = D // 2          # local attention feature dim (HH * HD)
N_CORES = 8
QH = 1024            # q-range processed per attention pass (psum budget)
NPAIR = HH // 2      # head pairs per core

DT_MODE = "bf16"

_CACHE = {}


def _build(dt_mode):
    import concourse.mybir as mybir
    from concourse import bacc
    from concourse.tile import TileContext
    from concourse.masks import make_upper_triangular

    F32 = mybir.dt.float32
    DT = mybir.dt.bfloat16 if dt_mode == "bf16" else mybir.dt.float32

    ADD = mybir.AluOpType.add
    MULT = mybir.AluOpType.mult
    EXP = mybir.ActivationFunctionType.Exp

    nc = bacc.Bacc("TRN2", target_bir_lowering=False, debug=False,
                   num_devices=N_CORES)

    xT = nc.dram_tensor("xT", [D, S], DT, kind="ExternalInput").ap()
    # pair-major weights: row block p*128..(p+1)*128 = [part, db, col] of pair p
    wq = nc.dram_tensor("wq", [NPAIR * 128, D], DT, kind="ExternalInput").ap()
    wk = nc.dram_tensor("wk", [NPAIR * 128, D], DT, kind="ExternalInput").ap()
    wv = nc.dram_tensor("wv", [NPAIR * 128, D], DT, kind="ExternalInput").ap()
    wo = nc.dram_tensor("wo", [DH, D], DT, kind="ExternalInput").ap()
    bq = nc.dram_tensor("bq", [128, NPAIR], F32, kind="ExternalInput").ap()
    bk = nc.dram_tensor("bk", [128, NPAIR], F32, kind="ExternalInput").ap()
    bv = nc.dram_tensor("bv", [128, DH], F32, kind="ExternalInput").ap()
    out = nc.dram_tensor("out", [S, D], F32, kind="ExternalOutput").ap()

    ND = D // 128        # 8 contraction tiles over D
    NS = S // 128        # 16 s-blocks
    NSC = S // 512       # 4 columns of 512 over S

    with TileContext(nc) as tc:
        with (
            tc.tile_pool(name="persist", bufs=1) as pp,
            tc.tile_pool(name="xt", bufs=ND) as pxt,
            tc.tile_pool(name="wqkv", bufs=3 * NPAIR) as pw,
            tc.tile_pool(name="wo", bufs=NPAIR) as pwo,
            tc.tile_pool(name="qz", bufs=HH) as pqz,
            tc.tile_pool(name="kT", bufs=NPAIR) as pkT,
            tc.tile_pool(name="vaug", bufs=NPAIR * NS) as pv,
            tc.tile_pool(name="attnT", bufs=NPAIR) as pattnT,
            tc.tile_pool(name="exp", bufs=4) as pexp,
            tc.tile_pool(name="au", bufs=3) as pau,
            tc.tile_pool(name="bcast", bufs=2) as pbc,
            tc.tile_pool(name="recip", bufs=2) as prc,
            tc.tile_pool(name="ostage", bufs=3) as post,
            tc.tile_pool(name="scps", bufs=2, space="PSUM") as pscps,
            tc.tile_pool(name="atps", bufs=1, space="PSUM") as patps,
            tc.tile_pool(name="fillps", bufs=2, space="PSUM") as pfill,
        ):
            # ---- input DMAs, most-urgent first; split across both rings ----
            # per-pair weight slabs [128, ND*128]; col db*128+c = w[db*128+part, p*128+c]
            xt_t = [pxt.tile([128, S], DT, tag="xt", name=f"xt{i}")
                    for i in range(ND)]
            wq_t = [pw.tile([128, D], DT, tag="wqkv", name=f"wq{p}")
                    for p in range(NPAIR)]
            wk_t = [pw.tile([128, D], DT, tag="wqkv", name=f"wk{p}")
                    for p in range(NPAIR)]
            wv_t = [pw.tile([128, D], DT, tag="wqkv", name=f"wv{p}")
                    for p in range(NPAIR)]
            wo_t = [pwo.tile([128, D], DT, tag="wo", name=f"wo{db}")
                    for db in range(NPAIR)]
            bq_t = pp.tile([128, NPAIR], F32, tag="bq")
            bk_t = pp.tile([128, NPAIR], F32, tag="bk")
            bv_t = pp.tile([128, DH], F32, tag="bv")
            SH = S // 2
            for db in range(ND):
                eng = nc.sync if db % 2 == 0 else nc.scalar
                eng.dma_start(xt_t[db][:, 0:SH],
                              xT[db * 128:(db + 1) * 128, 0:SH])
            nc.sync.dma_start(wq_t[0][:], wq[0:128, :])
            nc.scalar.dma_start(wk_t[0][:], wk[0:128, :])
            nc.sync.dma_start(bq_t[:], bq[:])
            nc.sync.dma_start(bk_t[:], bk[:])
            nc.scalar.dma_start(bv_t[:], bv[:])
            nc.scalar.dma_start(wv_t[0][:], wv[0:128, :])
            for db in range(ND):
                eng = nc.sync if db % 2 == 0 else nc.scalar
                eng.dma_start(xt_t[db][:, SH:S],
                              xT[db * 128:(db + 1) * 128, SH:S])
            nc.scalar.dma_start(wv_t[1][:], wv[128:256, :])
            for p in range(1, NPAIR):
                nc.sync.dma_start(wq_t[p][:], wq[p * 128:(p + 1) * 128, :])
                nc.sync.dma_start(wk_t[p][:], wk[p * 128:(p + 1) * 128, :])
            for p in (2, 3):
                nc.scalar.dma_start(wv_t[p][:], wv[p * 128:(p + 1) * 128, :])
            for db in range(NPAIR):
                nc.sync.dma_start(wo_t[db][:], wo[db * 128:(db + 1) * 128, :])

            # ---- constants ----
            ones_t = pp.tile([128, 2], F32, tag="ones")
            nc.gpsimd.memset(ones_t[:], 1.0)
            # causal mask for diagonal 128x128 squares of scoresT[k, q]:
            # valid (k <= q) <=> partition p <= free f -> upper-tri incl diag.
            mask_f = pp.tile([128, 128], F32, tag="maskf")
            make_upper_triangular(nc, mask_f[:], val=1.0, diag=True)
            mask_t = pp.tile([128, 128], DT, tag="mask")
            nc.vector.tensor_copy(mask_t[:], mask_f[:])

            # ---- persistent activations ----
            # qz[h]: zero-padded per-head q, transposed layout [128, S]; head h
            # occupies partitions hr..hr+64, rest stays zero so scores matmuls
            # run full-128-contraction (uniform PE mode, no retile drains).
            qz_t = [pqz.tile([128, S], DT, tag="qz", name=f"qz{h}")
                    for h in range(HH)]
            for h in range(HH):
                zr = 64 - (h % 2) * 64  # the complementary half
                nc.gpsimd.memset(qz_t[h][zr:zr + 64, :], 0.0)
            kT_t = [pkT.tile([128, S], DT, tag="kT", name=f"kT{p}")
                    for p in range(NPAIR)]
            # v_aug[p][sb]: [128, 2*65], per head [v(64) | ones]
            v_t = [[pv.tile([128, 2 * (HD + 1)], DT, tag="vaug",
                            name=f"vaug{p}_{sb}") for sb in range(NS)]
                   for p in range(NPAIR)]
            aT_t = [pattnT.tile([128, S], DT, tag="attnT", name=f"attnT{p}")
                    for p in range(NPAIR)]

            # ================= filler units =================
            def qk_units(p, scs):
                """q&k projection units for pair p over s-chunks scs."""
                units = []
                for scC in scs:
                    for proj in ("q", "k"):
                        w_t = (wq_t if proj == "q" else wk_t)[p]
                        holder = {}

                        def u1(p=p, scC=scC, proj=proj, w_t=w_t, holder=holder):
                            ps = pfill.tile([128, 512], F32, tag="fill",
                                            name=f"ps{proj}{p}_{scC}")
                            holder["ps"] = ps
                            for db in range(4):
                                nc.tensor.matmul(
                                    ps[:],
                                    lhsT=w_t[:, db * 128:(db + 1) * 128],
                                    rhs=xt_t[db][:, scC * 512:(scC + 1) * 512],
                                    start=(db == 0), stop=False,
                                )

                        def u2(p=p, scC=scC, proj=proj, w_t=w_t, holder=holder):
                            ps = holder["ps"]
                            for db in range(4, ND):
                                nc.tensor.matmul(
                                    ps[:],
                                    lhsT=w_t[:, db * 128:(db + 1) * 128],
                                    rhs=xt_t[db][:, scC * 512:(scC + 1) * 512],
                                    start=False, stop=(db == ND - 1),
                                )
                            cs = slice(scC * 512, (scC + 1) * 512)
                            if proj == "q":
                                nc.vector.tensor_scalar_add(
                                    qz_t[2 * p][0:64, cs], ps[0:64, :],
                                    bq_t[0:64, p:p + 1])
                                nc.vector.tensor_scalar_add(
                                    qz_t[2 * p + 1][64:128, cs], ps[64:128, :],
                                    bq_t[64:128, p:p + 1])
                            else:
                                nc.vector.tensor_scalar_add(
                                    kT_t[p][:, cs], ps[:], bk_t[:, p:p + 1])

                        units.append(u1)
                        units.append(u2)
                return units

            def v_units(p, sbs):
                """V projection for pair p: per s-block psum [128(s),128(vc)]."""
                units = []
                for sb in sbs:
                    def u(p=p, sb=sb):
                        ps = pfill.tile([128, 512], F32, tag="fill",
                                        name=f"psv{p}_{sb}")
                        for db in range(ND):
                            nc.tensor.matmul(
                                ps[:, 0:128],
                                lhsT=xt_t[db][:, sb * 128:(sb + 1) * 128],
                                rhs=wv_t[p][:, db * 128:(db + 1) * 128],
                                start=(db == 0), stop=(db == ND - 1),
                            )
                        vt = v_t[p][sb]
                        v3 = vt[:].rearrange("p (h e) -> p h e", e=HD + 1)
                        nc.vector.tensor_tensor(
                            v3[:, :, 0:HD],
                            ps[:, 0:128].rearrange("p (h e) -> p h e", e=HD),
                            bv_t[:, p * 128:(p + 1) * 128].rearrange(
                                "p (h e) -> p h e", e=HD),
                            op=ADD,
                        )
                        nc.vector.tensor_copy(
                            v3[:, :, HD:HD + 1],
                            ones_t[:].rearrange("p (h e) -> p h e", e=1),
                        )
                    units.append(u)
                return units

            def outproj_units(sb_list):
                units = []
                for sb in sb_list:
                    for jc in range(D // 512):
                        def u(sb=sb, jc=jc):
                            ps = pfill.tile([128, 512], F32, tag="fill",
                                            name=f"po{sb}_{jc}")
                            for db in range(NPAIR):
                                nc.tensor.matmul(
                                    ps[:],
                                    lhsT=aT_t[db][:, sb * 128:(sb + 1) * 128],
                                    rhs=wo_t[db][:, jc * 512:(jc + 1) * 512],
                                    start=(db == 0), stop=(db == NPAIR - 1),
                                )
                            ot = post.tile([128, 512], F32, tag="ostage",
                                           name=f"ot{sb}_{jc}")
                            nc.vector.tensor_copy(ot[:], ps[:])
                            nc.sync.dma_start(
                                out[sb * 128:(sb + 1) * 128,
                                    jc * 512:(jc + 1) * 512],
                                ot[:],
                            )
                        units.append(u)
                return units

            def make_popper(units, n_slots, skip_first=0):
                """Evenly pace `units` over `n_slots` popper() calls."""
                state = {"credit": 0.0, "slot": 0}
                rate = len(units) / max(n_slots - skip_first, 1)

                def popper():
                    state["slot"] += 1
                    if state["slot"] <= skip_first:
                        return
                    state["credit"] += rate
                    while units and state["credit"] >= 1.0:
                        state["credit"] -= 1.0
                        units.pop(0)()
                return popper, units

            # ================= attention =================
            def chunk_cols(lo):
                chunks = []
                c = lo
                while c < QH:
                    c1 = min((c // 512 + 1) * 512, QH)
                    chunks.append((c, c1))
                    c = c1
                return chunks

            pending_fin = []

            def emit_normalize(h, qh, at, c0, c1, defer=True):
                hb, hr = h // 2, (h % 2) * 64
                q0 = qh * QH
                w = c1 - c0
                au = pau.tile([65, w], F32, tag="au", name=f"au{h}_{qh}_{c0}")
                nc.vector.tensor_copy(au[:], at[0:65, c0:c1])
                dn = prc.tile([1, w], F32, tag="dn", name=f"dn{h}_{qh}_{c0}")
                nc.vector.tensor_copy(dn[:], au[64:65, :])
                rc = prc.tile([1, w], F32, tag="rc", name=f"rc{h}_{qh}_{c0}")
                nc.vector.reciprocal_approx_fast(rc[:], dn[:])
                bc = pbc.tile([64, w], F32, tag="bc", name=f"bc{h}_{qh}_{c0}")
                nc.gpsimd.partition_broadcast(bc[:], rc[:])

                def fin():
                    nc.vector.tensor_tensor(
                        aT_t[hb][hr:hr + 64, q0 + c0:q0 + c1],
                        au[0:64, :],
                        bc[:],
                        op=MULT,
                    )
                # defer the multiply into the next attention loop so the
                # bcast wait never head-blocks the DVE queue (masks behind
                # it gate the PE); flushed before any aT consumer emits.
                if defer:
                    pending_fin.append(fin)
                else:
                    fin()

            def attn_head_qh(h, qh, popper, split_at=None):
                hb, hr = h // 2, (h % 2) * 64
                vcol = (h % 2) * (HD + 1)
                q0 = qh * QH
                at = patps.tile([65, QH], F32, tag="at", name=f"at{h}_{qh}")
                nkb = (q0 + QH) // 128
                kb_split = ((q0 + split_at - 1) // 128 if split_at is not None
                            else None)

                def scores(kb):
                    k0 = kb * 128
                    lo = max(k0 - q0, 0)
                    sc = pscps.tile([128, QH], F32, tag="sc",
                                    name=f"sc{h}_{qh}_{kb}")
                    for (c0, c1) in chunk_cols(lo):
                        nc.tensor.matmul(
                            sc[:, c0:c1],
                            lhsT=kT_t[hb][:, k0:k0 + 128],
                            rhs=qz_t[h][:, q0 + c0:q0 + c1],
                            start=True, stop=True,
                        )
                    return sc

                def exp_pv(kb, sc):
                    k0 = kb * 128
                    lo = max(k0 - q0, 0)
                    et = pexp.tile([128, QH], DT, tag="exp",
                                   name=f"et{h}_{qh}_{kb}")
                    nc.scalar.activation(et[:, lo:QH], sc[:, lo:QH],
                                         EXP, scale=1.0 / np.sqrt(HD))
                    if k0 >= q0:
                        nc.vector.tensor_mul(et[:, lo:lo + 128],
                                             et[:, lo:lo + 128],
                                             mask_t[:])
                    for (c0, c1) in chunk_cols(lo):
                        nc.tensor.matmul(
                            at[0:65, c0:c1],
                            lhsT=v_t[hb][kb][:, vcol:vcol + HD + 1],
                            rhs=et[:, c0:c1],
                            start=(kb == 0),
                            stop=(kb == (q0 + c1 - 1) // 128),
                        )

                # software pipeline: scores one kb ahead of exp+pv; filler
                # units run between scores and pv so the PE queue never
                # head-blocks on ACT's exp.
                prev = scores(0)
                for kb in range(1, nkb):
                    cur = scores(kb)
                    popper()
                    exp_pv(kb - 1, prev)
                    if kb == 3:
                        while pending_fin:
                            pending_fin.pop(0)()
                    if kb_split is not None and kb - 1 == kb_split:
                        # first column chunk fully accumulated: normalize it
                        # now so consumers (output projection) start early.
                        emit_normalize(h, qh, at, 0, split_at, defer=False)
                    prev = cur
                popper()
                exp_pv(nkb - 1, prev)
                if kb_split is not None:
                    emit_normalize(h, qh, at, split_at, QH, defer=False)
                else:
                    emit_normalize(h, qh, at, 0, QH)
                popper()

            # ================= schedule =================
            def run_pair(p, units):
                n_slots = 2 * sum((qh * QH + QH) // 128
                                  for qh in range(S // QH))
                popper, _ = make_popper(units, n_slots)
                for h in (2 * p, 2 * p + 1):
                    for qh in range(S // QH):
                        attn_head_qh(h, qh, popper)
                while units:
                    units.pop(0)()

            # minimal slice of pair-0 QKV up front: q/k s-cols 0..1024 and
            # v s-blocks 0..7 are all head 0 qh=0 needs, and they only read
            # the first x column-half, which lands early.
            for u in qk_units(0, [0, 1]) + v_units(0, range(8)):
                u()
            attn_head_qh(0, 0, lambda: None)
            for u in qk_units(0, [2, 3]):
                u()
            rest0 = (v_units(0, range(8, NS)) + qk_units(1, [0, 1])
                     + v_units(1, range(8)) + qk_units(1, [2, 3])
                     + v_units(1, range(8, NS)))
            popper, _ = make_popper(rest0, 40)
            attn_head_qh(0, 1, popper)
            attn_head_qh(1, 0, popper)
            attn_head_qh(1, 1, popper)
            while rest0:
                rest0.pop(0)()

            # pairs 1..2: attention overlapped with next pair's QKV
            run_pair(1, qk_units(2, range(NSC)) + v_units(2, range(NS)))
            run_pair(2, qk_units(3, [0, 1]) + v_units(3, range(8)))

            # pair 3: finish own QKV during qh=0; output-project the first
            # s-half during qh=1 (gated so aT of both qh=0 heads lands);
            # split-normalize the qh=1 heads so sb8-11 can run in the tail
            # of h7/qh=1 instead of serializing after the last normalize.
            p = NPAIR - 1
            u_a = qk_units(p, [2, 3])
            popper, _ = make_popper(u_a, 8)
            attn_head_qh(2 * p, 0, popper)
            u_b = v_units(p, range(8, NS))
            popper, _ = make_popper(u_b, 8)
            attn_head_qh(2 * p + 1, 0, popper)

            while pending_fin:
                pending_fin.pop(0)()
            u_c = outproj_units(range(NS // 2))
            popper, _ = make_popper(u_c, 28, skip_first=2)
            attn_head_qh(2 * p, 1, popper, split_at=512)
            u_d = outproj_units(range(NS // 2, NS // 2 + 4))
            state = {"slot": 0}

            def popper_h7():
                state["slot"] += 1
                if u_c:
                    u_c.pop(0)()
                elif state["slot"] >= 13 and u_d:
                    u_d.pop(0)()
                    if u_d:
                        u_d.pop(0)()
            attn_head_qh(2 * p + 1, 1, popper_h7, split_at=512)
            for u in u_c + u_d:
                u()

            # tail: last quarter of the output projection
            for u in outproj_units(range(NS // 2 + 4, NS)):
                u()

    nc.compile()
    return nc


def _get_nc(dt_mode):
    if dt_mode not in _CACHE:
        _CACHE[dt_mode] = _build(dt_mode)
    return _CACHE[dt_mode]


def _pair_major(w):
    # [D, DH] -> [NPAIR*128, ND*128]: row p*128+part, col db*128+c
    #   = w[db*128+part, p*128+c]
    return np.ascontiguousarray(
        w.reshape(8, 128, NPAIR, 128).transpose(2, 1, 0, 3).reshape(
            NPAIR * 128, 1024))


def make_in_maps(x, Wq_w, Wq_b, Wk_w, Wk_b, Wv_w, Wv_b, Wo_w, Wo_b, np_dt):
    in_maps = []
    for core in range(N_CORES):
        b, half = core // 2, core % 2
        sl = slice(half * DH, (half + 1) * DH)
        in_maps.append({
            "xT": np.ascontiguousarray(x[b].T).astype(np_dt),
            "wq": _pair_major(Wq_w[:, sl]).astype(np_dt),
            "wk": _pair_major(Wk_w[:, sl]).astype(np_dt),
            "wv": _pair_major(Wv_w[:, sl]).astype(np_dt),
            "wo": np.ascontiguousarray(Wo_w[sl, :]).astype(np_dt),
            "bq": np.ascontiguousarray(Wq_b[sl].reshape(-1, 128).T),
            "bk": np.ascontiguousarray(Wk_b[sl].reshape(-1, 128).T),
            "bv": np.broadcast_to(Wv_b[sl], (128, DH)).copy(),
        })
    return in_maps


def kernel(x, Wq_w, Wq_b, Wk_w, Wk_b, Wv_w, Wv_b, Wo_w, Wo_b):
    from concourse.bass_utils import run_bass_kernel_spmd

    np_dt = ml_dtypes.bfloat16 if DT_MODE == "bf16" else np.float32

    args = [np.asarray(a, np.float32) for a in
            (x, Wq_w, Wq_b, Wk_w, Wk_b, Wv_w, Wv_b, Wo_w, Wo_b)]
    x, Wq_w, Wq_b, Wk_w, Wk_b, Wv_w, Wv_b, Wo_w, Wo_b = args

    nc = _get_nc(DT_MODE)
    in_maps = make_in_maps(x, Wq_w, Wq_b, Wk_w, Wk_b, Wv_w, Wv_b, Wo_w, Wo_b,
                           np_dt)
    res = run_bass_kernel_spmd(nc, in_maps, list(range(N_CORES)))

    out = np.empty((B, S, D), np.float32)
    for b in range(B):
        out[b] = res.results[2 * b]["out"] + res.results[2 * b + 1]["out"] + Wo_b
    return out


# revision 24
# speedup vs baseline: 1.0059x; 1.0059x over previous
"""Trainium2 Bass kernel for nn_MultiHeadAttention (B=4, S=2048, D=1024, H=16).

Sharding: 8 cores = batch(4) x head-half(2).  Each core computes, for its
batch element, 8 of the 16 heads: QKV projections against column-sliced
weights, causal attention, and the output projection against the matching
row-slice of Wo.  The two partial outputs per batch element are summed on
the host (replaces the tensor-parallel all-reduce), and Wo_b is added there.

v3: single fused pipeline.  The QKV projections of the next head-pair and
the output projection are interleaved into the attention kb-loop as PE
"filler" units, so the tensor engine keeps streaming while the scalar
engine runs exp (an idle PE lets the HAM clock gate throttle it to
1.2 GHz).  All matmuls use the uniform (128,128) PE tiling mode (q is
stored zero-padded per head).  Softmax normalization avoids GpSimd
entirely: one DVE copy frees the PSUM accumulator, a stride-0-source DMA
broadcasts the denominator row across 64 partitions, and DVE does
reciprocal + multiply.  Weights arrive pair-major so each pair's slice is
one contiguous DMA, with the x tiles split across both HW DMA rings.
"""

import sys

if "/opt/trn_rl_repo" not in sys.path:
    sys.path.insert(0, "/opt/trn_rl_repo")

import numpy as np
import ml_dtypes

B, S, D = 4, 2048, 1024
H, HD = 16, 64
HH = H // 2          # heads per core
DH = D // 2          # local attention feature dim (HH * HD)
N_CORES = 8
QH = 1024            # q-range processed per attention pass (psum budget)
NPAIR = HH // 2      # head pairs per core

DT_MODE = "bf16"

_CACHE = {}


def _build(dt_mode):
    import concourse.mybir as mybir
    from concourse import bacc
    from concourse.tile import TileContext
    from concourse.masks import make_upper_triangular

    F32 = mybir.dt.float32
    DT = mybir.dt.bfloat16 if dt_mode == "bf16" else mybir.dt.float32

    ADD = mybir.AluOpType.add
    MULT = mybir.AluOpType.mult
    EXP = mybir.ActivationFunctionType.Exp

    nc = bacc.Bacc("TRN2", target_bir_lowering=False, debug=False,
                   num_devices=N_CORES)

    xT = nc.dram_tensor("xT", [D, S], DT, kind="ExternalInput").ap()
    # pair-major weights: row block p*128..(p+1)*128 = [part, db, col] of pair p
    wq = nc.dram_tensor("wq", [NPAIR * 128, D], DT, kind="ExternalInput").ap()
    wk = nc.dram_tensor("wk", [NPAIR * 128, D], DT, kind="ExternalInput").ap()
    wv = nc.dram_tensor("wv", [NPAIR * 128, D], DT, kind="ExternalInput").ap()
    wo = nc.dram_tensor("wo", [DH, D], DT, kind="ExternalInput").ap()
    bq = nc.dram_tensor("bq", [128, NPAIR], F32, kind="ExternalInput").ap()
    bk = nc.dram_tensor("bk", [128, NPAIR], F32, kind="ExternalInput").ap()
    bv = nc.dram_tensor("bv", [128, DH], F32, kind="ExternalInput").ap()
    out = nc.dram_tensor("out", [S, D], F32, kind="ExternalOutput").ap()

    ND = D // 128        # 8 contraction tiles over D
    NS = S // 128        # 16 s-blocks
    NSC = S // 512       # 4 columns of 512 over S

    with TileContext(nc) as tc:
        with (
            tc.tile_pool(name="persist", bufs=1) as pp,
            tc.tile_pool(name="xt", bufs=ND) as pxt,
            tc.tile_pool(name="wqkv", bufs=3 * NPAIR) as pw,
            tc.tile_pool(name="wo", bufs=NPAIR) as pwo,
            tc.tile_pool(name="qz", bufs=HH) as pqz,
            tc.tile_pool(name="kT", bufs=NPAIR) as pkT,
            tc.tile_pool(name="vaug", bufs=NPAIR * NS) as pv,
            tc.tile_pool(name="attnT", bufs=NPAIR) as pattnT,
            tc.tile_pool(name="exp", bufs=4) as pexp,
            tc.tile_pool(name="au", bufs=3) as pau,
            tc.tile_pool(name="bcast", bufs=2) as pbc,
            tc.tile_pool(name="recip", bufs=2) as prc,
            tc.tile_pool(name="ostage", bufs=3) as post,
            tc.tile_pool(name="scps", bufs=2, space="PSUM") as pscps,
            tc.tile_pool(name="atps", bufs=1, space="PSUM") as patps,
            tc.tile_pool(name="fillps", bufs=2, space="PSUM") as pfill,
        ):
            # ---- input DMAs, most-urgent first; split across both rings ----
            # per-pair weight slabs [128, ND*128]; col db*128+c = w[db*128+part, p*128+c]
            xt_t = [pxt.tile([128, S], DT, tag="xt", name=f"xt{i}")
                    for i in range(ND)]
            wq_t = [pw.tile([128, D], DT, tag="wqkv", name=f"wq{p}")
                    for p in range(NPAIR)]
            wk_t = [pw.tile([128, D], DT, tag="wqkv", name=f"wk{p}")
                    for p in range(NPAIR)]
            wv_t = [pw.tile([128, D], DT, tag="wqkv", name=f"wv{p}")
                    for p in range(NPAIR)]
            wo_t = [pwo.tile([128, D], DT, tag="wo", name=f"wo{db}")
                    for db in range(NPAIR)]
            bq_t = pp.tile([128, NPAIR], F32, tag="bq")
            bk_t = pp.tile([128, NPAIR], F32, tag="bk")
            bv_t = pp.tile([128, DH], F32, tag="bv")
            SH = S // 2
            for db in range(ND):
                eng = nc.sync if db % 2 == 0 else nc.scalar
                eng.dma_start(xt_t[db][:, 0:SH],
                              xT[db * 128:(db + 1) * 128, 0:SH])
            nc.sync.dma_start(wq_t[0][:], wq[0:128, :])
            nc.scalar.dma_start(wk_t[0][:], wk[0:128, :])
            nc.sync.dma_start(bq_t[:], bq[:])
            nc.sync.dma_start(bk_t[:], bk[:])
            nc.scalar.dma_start(bv_t[:], bv[:])
            nc.scalar.dma_start(wv_t[0][:], wv[0:128, :])
            for db in range(ND):
                eng = nc.sync if db % 2 == 0 else nc.scalar
                eng.dma_start(xt_t[db][:, SH:S],
                              xT[db * 128:(db + 1) * 128, SH:S])
            nc.scalar.dma_start(wv_t[1][:], wv[128:256, :])
            for p in range(1, NPAIR):
                nc.sync.dma_start(wq_t[p][:], wq[p * 128:(p + 1) * 128, :])
                nc.sync.dma_start(wk_t[p][:], wk[p * 128:(p + 1) * 128, :])
            for p in (2, 3):
                nc.scalar.dma_start(wv_t[p][:], wv[p * 128:(p + 1) * 128, :])
            for db in range(NPAIR):
                nc.sync.dma_start(wo_t[db][:], wo[db * 128:(db + 1) * 128, :])

            # ---- constants ----
            ones_t = pp.tile([128, 2], F32, tag="ones")
            nc.gpsimd.memset(ones_t[:], 1.0)
            # causal mask for diagonal 128x128 squares of scoresT[k, q]:
            # valid (k <= q) <=> partition p <= free f -> upper-tri incl diag.
            mask_f = pp.tile([128, 128], F32, tag="maskf")
            make_upper_triangular(nc, mask_f[:], val=1.0, diag=True)
            mask_t = pp.tile([128, 128], DT, tag="mask")
            nc.vector.tensor_copy(mask_t[:], mask_f[:])

            # ---- persistent activations ----
            # qz[h]: zero-padded per-head q, transposed layout [128, S]; head h
            # occupies partitions hr..hr+64, rest stays zero so scores matmuls
            # run full-128-contraction (uniform PE mode, no retile drains).
            qz_t = [pqz.tile([128, S], DT, tag="qz", name=f"qz{h}")
                    for h in range(HH)]
            for h in range(HH):
                zr = 64 - (h % 2) * 64  # the complementary half
                nc.gpsimd.memset(qz_t[h][zr:zr + 64, :], 0.0)
            kT_t = [pkT.tile([128, S], DT, tag="kT", name=f"kT{p}")
                    for p in range(NPAIR)]
            # v_aug[p][sb]: [128, 2*65], per head [v(64) | ones]
            v_t = [[pv.tile([128, 2 * (HD + 1)], DT, tag="vaug",
                            name=f"vaug{p}_{sb}") for sb in range(NS)]
                   for p in range(NPAIR)]
            aT_t = [pattnT.tile([128, S], DT, tag="attnT", name=f"attnT{p}")
                    for p in range(NPAIR)]

            # ================= filler units =================
            def qk_units(p, scs):
                """q&k projection units for pair p over s-chunks scs."""
                units = []
                for scC in scs:
                    for proj in ("q", "k"):
                        w_t = (wq_t if proj == "q" else wk_t)[p]
                        holder = {}

                        def u1(p=p, scC=scC, proj=proj, w_t=w_t, holder=holder):
                            ps = pfill.tile([128, 512], F32, tag="fill",
                                            name=f"ps{proj}{p}_{scC}")
                            holder["ps"] = ps
                            for db in range(4):
                                nc.tensor.matmul(
                                    ps[:],
                                    lhsT=w_t[:, db * 128:(db + 1) * 128],
                                    rhs=xt_t[db][:, scC * 512:(scC + 1) * 512],
                                    start=(db == 0), stop=False,
                                )

                        def u2(p=p, scC=scC, proj=proj, w_t=w_t, holder=holder):
                            ps = holder["ps"]
                            for db in range(4, ND):
                                nc.tensor.matmul(
                                    ps[:],
                                    lhsT=w_t[:, db * 128:(db + 1) * 128],
                                    rhs=xt_t[db][:, scC * 512:(scC + 1) * 512],
                                    start=False, stop=(db == ND - 1),
                                )
                            cs = slice(scC * 512, (scC + 1) * 512)
                            if proj == "q":
                                nc.vector.tensor_scalar_add(
                                    qz_t[2 * p][0:64, cs], ps[0:64, :],
                                    bq_t[0:64, p:p + 1])
                                nc.vector.tensor_scalar_add(
                                    qz_t[2 * p + 1][64:128, cs], ps[64:128, :],
                                    bq_t[64:128, p:p + 1])
                            else:
                                nc.vector.tensor_scalar_add(
                                    kT_t[p][:, cs], ps[:], bk_t[:, p:p + 1])

                        units.append(u1)
                        units.append(u2)
                return units

            def v_units(p, sbs):
                """V projection for pair p: per s-block psum [128(s),128(vc)]."""
                units = []
                for sb in sbs:
                    def u(p=p, sb=sb):
                        ps = pfill.tile([128, 512], F32, tag="fill",
                                        name=f"psv{p}_{sb}")
                        for db in range(ND):
                            nc.tensor.matmul(
                                ps[:, 0:128],
                                lhsT=xt_t[db][:, sb * 128:(sb + 1) * 128],
                                rhs=wv_t[p][:, db * 128:(db + 1) * 128],
                                start=(db == 0), stop=(db == ND - 1),
                            )
                        vt = v_t[p][sb]
                        v3 = vt[:].rearrange("p (h e) -> p h e", e=HD + 1)
                        nc.vector.tensor_tensor(
                            v3[:, :, 0:HD],
                            ps[:, 0:128].rearrange("p (h e) -> p h e", e=HD),
                            bv_t[:, p * 128:(p + 1) * 128].rearrange(
                                "p (h e) -> p h e", e=HD),
                            op=ADD,
                        )
                        nc.vector.tensor_copy(
                            v3[:, :, HD:HD + 1],
                            ones_t[:].rearrange("p (h e) -> p h e", e=1),
                        )
                    units.append(u)
                return units

            def outproj_units(sb_list):
                units = []
                for sb in sb_list:
                    for jc in range(D // 512):
                        def u(sb=sb, jc=jc):
                            ps = pfill.tile([128, 512], F32, tag="fill",
                                            name=f"po{sb}_{jc}")
                            for db in range(NPAIR):
                                nc.tensor.matmul(
                                    ps[:],
                                    lhsT=aT_t[db][:, sb * 128:(sb + 1) * 128],
                                    rhs=wo_t[db][:, jc * 512:(jc + 1) * 512],
                                    start=(db == 0), stop=(db == NPAIR - 1),
                                )
                            ot = post.tile([128, 512], F32, tag="ostage",
                                           name=f"ot{sb}_{jc}")
                            nc.vector.tensor_copy(ot[:], ps[:])
                            nc.sync.dma_start(
                                out[sb * 128:(sb + 1) * 128,
                                    jc * 512:(jc + 1) * 512],
                                ot[:],
                            )
                        units.append(u)
                return units

            def make_popper(units, n_slots, skip_first=0):
                """Evenly pace `units` over `n_slots` popper() calls."""
                state = {"credit": 0.0, "slot": 0}
                rate = len(units) / max(n_slots - skip_first, 1)

                def popper():
                    state["slot"] += 1
                    if state["slot"] <= skip_first:
                        return
                    state["credit"] += rate
                    while units and state["credit"] >= 1.0:
                        state["credit"] -= 1.0
                        units.pop(0)()
                return popper, units

            # ================= attention =================
            def chunk_cols(lo):
                chunks = []
                c = lo
                while c < QH:
                    c1 = min((c // 512 + 1) * 512, QH)
                    chunks.append((c, c1))
                    c = c1
                return chunks

            pending_fin = []

            def emit_normalize(h, qh, at, c0, c1, defer=True):
                hb, hr = h // 2, (h % 2) * 64
                q0 = qh * QH
                w = c1 - c0
                au = pau.tile([65, w], F32, tag="au", name=f"au{h}_{qh}_{c0}")
                nc.scalar.copy(au[:], at[0:65, c0:c1])
                dn = prc.tile([1, w], F32, tag="dn", name=f"dn{h}_{qh}_{c0}")
                nc.vector.tensor_copy(dn[:], au[64:65, :])
                rc = prc.tile([1, w], F32, tag="rc", name=f"rc{h}_{qh}_{c0}")
                nc.vector.reciprocal_approx_fast(rc[:], dn[:])
                bc = pbc.tile([64, w], F32, tag="bc", name=f"bc{h}_{qh}_{c0}")
                nc.gpsimd.partition_broadcast(bc[:], rc[:])

                def fin():
                    nc.vector.tensor_tensor(
                        aT_t[hb][hr:hr + 64, q0 + c0:q0 + c1],
                        au[0:64, :],
                        bc[:],
                        op=MULT,
                    )
                # defer the multiply into the next attention loop so the
                # bcast wait never head-blocks the DVE queue (masks behind
                # it gate the PE); flushed before any aT consumer emits.
                if defer:
                    pending_fin.append(fin)
                else:
                    fin()

            def attn_head_qh(h, qh, popper, split_at=None):
                hb, hr = h // 2, (h % 2) * 64
                vcol = (h % 2) * (HD + 1)
                q0 = qh * QH
                at = patps.tile([65, QH], F32, tag="at", name=f"at{h}_{qh}")
                nkb = (q0 + QH) // 128
                kb_split = ((q0 + split_at - 1) // 128 if split_at is not None
                            else None)

                def scores(kb):
                    k0 = kb * 128
                    lo = max(k0 - q0, 0)
                    sc = pscps.tile([128, QH], F32, tag="sc",
                                    name=f"sc{h}_{qh}_{kb}")
                    for (c0, c1) in chunk_cols(lo):
                        nc.tensor.matmul(
                            sc[:, c0:c1],
                            lhsT=kT_t[hb][:, k0:k0 + 128],
                            rhs=qz_t[h][:, q0 + c0:q0 + c1],
                            start=True, stop=True,
                        )
                    return sc

                def exp_pv(kb, sc):
                    k0 = kb * 128
                    lo = max(k0 - q0, 0)
                    et = pexp.tile([128, QH], DT, tag="exp",
                                   name=f"et{h}_{qh}_{kb}")
                    nc.scalar.activation(et[:, lo:QH], sc[:, lo:QH],
                                         EXP, scale=1.0 / np.sqrt(HD))
                    if k0 >= q0:
                        nc.vector.tensor_mul(et[:, lo:lo + 128],
                                             et[:, lo:lo + 128],
                                             mask_t[:])
                    for (c0, c1) in chunk_cols(lo):
                        nc.tensor.matmul(
                            at[0:65, c0:c1],
                            lhsT=v_t[hb][kb][:, vcol:vcol + HD + 1],
                            rhs=et[:, c0:c1],
                            start=(kb == 0),
                            stop=(kb == (q0 + c1 - 1) // 128),
                        )

                # software pipeline: scores one kb ahead of exp+pv; filler
                # units run between scores and pv so the PE queue never
                # head-blocks on ACT's exp.
                prev = scores(0)
                for kb in range(1, nkb):
                    cur = scores(kb)
                    popper()
                    exp_pv(kb - 1, prev)
                    if kb == 3:
                        while pending_fin:
                            pending_fin.pop(0)()
                    if kb_split is not None and kb - 1 == kb_split:
                        # first column chunk fully accumulated: normalize it
                        # now so consumers (output projection) start early.
                        emit_normalize(h, qh, at, 0, split_at, defer=False)
                    prev = cur
                popper()
                exp_pv(nkb - 1, prev)
                if kb_split is not None:
                    emit_normalize(h, qh, at, split_at, QH, defer=False)
                else:
                    emit_normalize(h, qh, at, 0, QH)
                popper()

            # ================= schedule =================
            def run_pair(p, units):
                n_slots = 2 * sum((qh * QH + QH) // 128
                                  for qh in range(S // QH))
                popper, _ = make_popper(units, n_slots)
                for h in (2 * p, 2 * p + 1):
                    for qh in range(S // QH):
                        attn_head_qh(h, qh, popper)
                while units:
                    units.pop(0)()

            # minimal slice of pair-0 QKV up front: q/k s-cols 0..1024 and
            # v s-blocks 0..7 are all head 0 qh=0 needs, and they only read
            # the first x column-half, which lands early.
            for u in qk_units(0, [0, 1]) + v_units(0, range(8)):
                u()
            attn_head_qh(0, 0, lambda: None)
            for u in qk_units(0, [2, 3]):
                u()
            rest0 = (v_units(0, range(8, NS)) + qk_units(1, [0, 1])
                     + v_units(1, range(8)) + qk_units(1, [2, 3])
                     + v_units(1, range(8, NS)))
            popper, _ = make_popper(rest0, 40)
            attn_head_qh(0, 1, popper)
            attn_head_qh(1, 0, popper)
            attn_head_qh(1, 1, popper)
            while rest0:
                rest0.pop(0)()

            # pairs 1..2: attention overlapped with next pair's QKV
            run_pair(1, qk_units(2, range(NSC)) + v_units(2, range(NS)))
            run_pair(2, qk_units(3, [0, 1]) + v_units(3, range(8)))

            # pair 3: finish own QKV during qh=0; output-project the first
            # s-half during qh=1 (gated so aT of both qh=0 heads lands);
            # split-normalize the qh=1 heads so sb8-11 can run in the tail
            # of h7/qh=1 instead of serializing after the last normalize.
            p = NPAIR - 1
            u_a = qk_units(p, [2, 3])
            popper, _ = make_popper(u_a, 8)
            attn_head_qh(2 * p, 0, popper)
            u_b = v_units(p, range(8, NS))
            popper, _ = make_popper(u_b, 8)
            attn_head_qh(2 * p + 1, 0, popper)

            while pending_fin:
                pending_fin.pop(0)()
            u_c = outproj_units(range(NS // 2))
            popper, _ = make_popper(u_c, 28, skip_first=2)
            attn_head_qh(2 * p, 1, popper, split_at=512)
            u_d = outproj_units(range(NS // 2, NS // 2 + 4))
            state = {"slot": 0}

            def popper_h7():
                state["slot"] += 1
                if u_c:
                    u_c.pop(0)()
                elif state["slot"] >= 13 and u_d:
                    u_d.pop(0)()
                    if u_d:
                        u_d.pop(0)()
            attn_head_qh(2 * p + 1, 1, popper_h7, split_at=512)
            for u in u_c + u_d:
                u()

            # tail: last quarter of the output projection
            for u in outproj_units(range(NS // 2 + 4, NS)):
                u()

    nc.compile()
    return nc


def _get_nc(dt_mode):
    if dt_mode not in _CACHE:
        _CACHE[dt_mode] = _build(dt_mode)
    return _CACHE[dt_mode]


def _pair_major(w):
    # [D, DH] -> [NPAIR*128, ND*128]: row p*128+part, col db*128+c
    #   = w[db*128+part, p*128+c]
    return np.ascontiguousarray(
        w.reshape(8, 128, NPAIR, 128).transpose(2, 1, 0, 3).reshape(
            NPAIR * 128, 1024))


def make_in_maps(x, Wq_w, Wq_b, Wk_w, Wk_b, Wv_w, Wv_b, Wo_w, Wo_b, np_dt):
    in_maps = []
    for core in range(N_CORES):
        b, half = core // 2, core % 2
        sl = slice(half * DH, (half + 1) * DH)
        in_maps.append({
            "xT": np.ascontiguousarray(x[b].T).astype(np_dt),
            "wq": _pair_major(Wq_w[:, sl]).astype(np_dt),
            "wk": _pair_major(Wk_w[:, sl]).astype(np_dt),
            "wv": _pair_major(Wv_w[:, sl]).astype(np_dt),
            "wo": np.ascontiguousarray(Wo_w[sl, :]).astype(np_dt),
            "bq": np.ascontiguousarray(Wq_b[sl].reshape(-1, 128).T),
            "bk": np.ascontiguousarray(Wk_b[sl].reshape(-1, 128).T),
            "bv": np.broadcast_to(Wv_b[sl], (128, DH)).copy(),
        })
    return in_maps


def kernel(x, Wq_w, Wq_b, Wk_w, Wk_b, Wv_w, Wv_b, Wo_w, Wo_b):
    from concourse.bass_utils import run_bass_kernel_spmd

    np_dt = ml_dtypes.bfloat16 if DT_MODE == "bf16" else np.float32

    args = [np.asarray(a, np.float32) for a in
            (x, Wq_w, Wq_b, Wk_w, Wk_b, Wv_w, Wv_b, Wo_w, Wo_b)]
    x, Wq_w, Wq_b, Wk_w, Wk_b, Wv_w, Wv_b, Wo_w, Wo_b = args

    nc = _get_nc(DT_MODE)
    in_maps = make_in_maps(x, Wq_w, Wq_b, Wk_w, Wk_b, Wv_w, Wv_b, Wo_w, Wo_b,
                           np_dt)
    res = run_bass_kernel_spmd(nc, in_maps, list(range(N_CORES)))

    out = np.empty((B, S, D), np.float32)
    for b in range(B):
        out[b] = res.results[2 * b]["out"] + res.results[2 * b + 1]["out"] + Wo_b
    return out


# revision 25
# speedup vs baseline: 1.0077x; 1.0018x over previous
"""Trainium2 Bass kernel for nn_MultiHeadAttention (B=4, S=2048, D=1024, H=16).

Sharding: 8 cores = batch(4) x head-half(2).  Each core computes, for its
batch element, 8 of the 16 heads: QKV projections against column-sliced
weights, causal attention, and the output projection against the matching
row-slice of Wo.  The two partial outputs per batch element are summed on
the host (replaces the tensor-parallel all-reduce), and Wo_b is added there.

v3: single fused pipeline.  The QKV projections of the next head-pair and
the output projection are interleaved into the attention kb-loop as PE
"filler" units, so the tensor engine keeps streaming while the scalar
engine runs exp (an idle PE lets the HAM clock gate throttle it to
1.2 GHz).  All matmuls use the uniform (128,128) PE tiling mode (q is
stored zero-padded per head).  Softmax normalization avoids GpSimd
entirely: one DVE copy frees the PSUM accumulator, a stride-0-source DMA
broadcasts the denominator row across 64 partitions, and DVE does
reciprocal + multiply.  Weights arrive pair-major so each pair's slice is
one contiguous DMA, with the x tiles split across both HW DMA rings.
"""

import sys

if "/opt/trn_rl_repo" not in sys.path:
    sys.path.insert(0, "/opt/trn_rl_repo")

import numpy as np
import ml_dtypes

B, S, D = 4, 2048, 1024
H, HD = 16, 64
HH = H // 2          # heads per core
DH = D // 2          # local attention feature dim (HH * HD)
N_CORES = 8
QH = 1024            # q-range processed per attention pass (psum budget)
NPAIR = HH // 2      # head pairs per core

DT_MODE = "bf16"

_CACHE = {}


def _build(dt_mode):
    import concourse.mybir as mybir
    from concourse import bacc
    from concourse.tile import TileContext
    from concourse.masks import make_upper_triangular

    F32 = mybir.dt.float32
    DT = mybir.dt.bfloat16 if dt_mode == "bf16" else mybir.dt.float32

    ADD = mybir.AluOpType.add
    MULT = mybir.AluOpType.mult
    EXP = mybir.ActivationFunctionType.Exp

    nc = bacc.Bacc("TRN2", target_bir_lowering=False, debug=False,
                   num_devices=N_CORES)

    xT = nc.dram_tensor("xT", [D, S], DT, kind="ExternalInput").ap()
    # pair-major weights: row block p*128..(p+1)*128 = [part, db, col] of pair p
    wq = nc.dram_tensor("wq", [NPAIR * 128, D], DT, kind="ExternalInput").ap()
    wk = nc.dram_tensor("wk", [NPAIR * 128, D], DT, kind="ExternalInput").ap()
    wv = nc.dram_tensor("wv", [NPAIR * 128, D], DT, kind="ExternalInput").ap()
    wo = nc.dram_tensor("wo", [DH, D], DT, kind="ExternalInput").ap()
    bq = nc.dram_tensor("bq", [128, NPAIR], F32, kind="ExternalInput").ap()
    bk = nc.dram_tensor("bk", [128, NPAIR], F32, kind="ExternalInput").ap()
    bv = nc.dram_tensor("bv", [128, DH], F32, kind="ExternalInput").ap()
    out = nc.dram_tensor("out", [S, D], F32, kind="ExternalOutput").ap()

    ND = D // 128        # 8 contraction tiles over D
    NS = S // 128        # 16 s-blocks
    NSC = S // 512       # 4 columns of 512 over S

    with TileContext(nc) as tc:
        with (
            tc.tile_pool(name="persist", bufs=1) as pp,
            tc.tile_pool(name="xt", bufs=ND) as pxt,
            tc.tile_pool(name="wqkv", bufs=3 * NPAIR) as pw,
            tc.tile_pool(name="wo", bufs=NPAIR) as pwo,
            tc.tile_pool(name="qz", bufs=HH) as pqz,
            tc.tile_pool(name="kT", bufs=NPAIR) as pkT,
            tc.tile_pool(name="vaug", bufs=NPAIR * NS) as pv,
            tc.tile_pool(name="attnT", bufs=NPAIR) as pattnT,
            tc.tile_pool(name="exp", bufs=5) as pexp,
            tc.tile_pool(name="au", bufs=3) as pau,
            tc.tile_pool(name="bcast", bufs=2) as pbc,
            tc.tile_pool(name="recip", bufs=2) as prc,
            tc.tile_pool(name="ostage", bufs=3) as post,
            tc.tile_pool(name="scps", bufs=2, space="PSUM") as pscps,
            tc.tile_pool(name="atps", bufs=1, space="PSUM") as patps,
            tc.tile_pool(name="fillps", bufs=2, space="PSUM") as pfill,
        ):
            # ---- input DMAs, most-urgent first; split across both rings ----
            # per-pair weight slabs [128, ND*128]; col db*128+c = w[db*128+part, p*128+c]
            xt_t = [pxt.tile([128, S], DT, tag="xt", name=f"xt{i}")
                    for i in range(ND)]
            wq_t = [pw.tile([128, D], DT, tag="wqkv", name=f"wq{p}")
                    for p in range(NPAIR)]
            wk_t = [pw.tile([128, D], DT, tag="wqkv", name=f"wk{p}")
                    for p in range(NPAIR)]
            wv_t = [pw.tile([128, D], DT, tag="wqkv", name=f"wv{p}")
                    for p in range(NPAIR)]
            wo_t = [pwo.tile([128, D], DT, tag="wo", name=f"wo{db}")
                    for db in range(NPAIR)]
            bq_t = pp.tile([128, NPAIR], F32, tag="bq")
            bk_t = pp.tile([128, NPAIR], F32, tag="bk")
            bv_t = pp.tile([128, DH], F32, tag="bv")
            SH = S // 2
            for db in range(ND):
                eng = nc.sync if db % 2 == 0 else nc.scalar
                eng.dma_start(xt_t[db][:, 0:SH],
                              xT[db * 128:(db + 1) * 128, 0:SH])
            nc.sync.dma_start(wq_t[0][:], wq[0:128, :])
            nc.scalar.dma_start(wk_t[0][:], wk[0:128, :])
            nc.sync.dma_start(bq_t[:], bq[:])
            nc.sync.dma_start(bk_t[:], bk[:])
            nc.scalar.dma_start(bv_t[:], bv[:])
            nc.scalar.dma_start(wv_t[0][:], wv[0:128, :])
            for db in range(ND):
                eng = nc.sync if db % 2 == 0 else nc.scalar
                eng.dma_start(xt_t[db][:, SH:S],
                              xT[db * 128:(db + 1) * 128, SH:S])
            nc.scalar.dma_start(wv_t[1][:], wv[128:256, :])
            for p in range(1, NPAIR):
                nc.sync.dma_start(wq_t[p][:], wq[p * 128:(p + 1) * 128, :])
                nc.sync.dma_start(wk_t[p][:], wk[p * 128:(p + 1) * 128, :])
            for p in (2, 3):
                nc.scalar.dma_start(wv_t[p][:], wv[p * 128:(p + 1) * 128, :])
            for db in range(NPAIR):
                nc.sync.dma_start(wo_t[db][:], wo[db * 128:(db + 1) * 128, :])

            # ---- constants ----
            ones_t = pp.tile([128, 2], F32, tag="ones")
            nc.gpsimd.memset(ones_t[:], 1.0)
            # causal mask for diagonal 128x128 squares of scoresT[k, q]:
            # valid (k <= q) <=> partition p <= free f -> upper-tri incl diag.
            mask_f = pp.tile([128, 128], F32, tag="maskf")
            make_upper_triangular(nc, mask_f[:], val=1.0, diag=True)
            mask_t = pp.tile([128, 128], DT, tag="mask")
            nc.vector.tensor_copy(mask_t[:], mask_f[:])

            # ---- persistent activations ----
            # qz[h]: zero-padded per-head q, transposed layout [128, S]; head h
            # occupies partitions hr..hr+64, rest stays zero so scores matmuls
            # run full-128-contraction (uniform PE mode, no retile drains).
            qz_t = [pqz.tile([128, S], DT, tag="qz", name=f"qz{h}")
                    for h in range(HH)]
            for h in range(HH):
                zr = 64 - (h % 2) * 64  # the complementary half
                nc.gpsimd.memset(qz_t[h][zr:zr + 64, :], 0.0)
            kT_t = [pkT.tile([128, S], DT, tag="kT", name=f"kT{p}")
                    for p in range(NPAIR)]
            # v_aug[p][sb]: [128, 2*65], per head [v(64) | ones]
            v_t = [[pv.tile([128, 2 * (HD + 1)], DT, tag="vaug",
                            name=f"vaug{p}_{sb}") for sb in range(NS)]
                   for p in range(NPAIR)]
            aT_t = [pattnT.tile([128, S], DT, tag="attnT", name=f"attnT{p}")
                    for p in range(NPAIR)]

            # ================= filler units =================
            def qk_units(p, scs):
                """q&k projection units for pair p over s-chunks scs."""
                units = []
                for scC in scs:
                    for proj in ("q", "k"):
                        w_t = (wq_t if proj == "q" else wk_t)[p]
                        holder = {}

                        def u1(p=p, scC=scC, proj=proj, w_t=w_t, holder=holder):
                            ps = pfill.tile([128, 512], F32, tag="fill",
                                            name=f"ps{proj}{p}_{scC}")
                            holder["ps"] = ps
                            for db in range(4):
                                nc.tensor.matmul(
                                    ps[:],
                                    lhsT=w_t[:, db * 128:(db + 1) * 128],
                                    rhs=xt_t[db][:, scC * 512:(scC + 1) * 512],
                                    start=(db == 0), stop=False,
                                )

                        def u2(p=p, scC=scC, proj=proj, w_t=w_t, holder=holder):
                            ps = holder["ps"]
                            for db in range(4, ND):
                                nc.tensor.matmul(
                                    ps[:],
                                    lhsT=w_t[:, db * 128:(db + 1) * 128],
                                    rhs=xt_t[db][:, scC * 512:(scC + 1) * 512],
                                    start=False, stop=(db == ND - 1),
                                )
                            cs = slice(scC * 512, (scC + 1) * 512)
                            if proj == "q":
                                nc.vector.tensor_scalar_add(
                                    qz_t[2 * p][0:64, cs], ps[0:64, :],
                                    bq_t[0:64, p:p + 1])
                                nc.vector.tensor_scalar_add(
                                    qz_t[2 * p + 1][64:128, cs], ps[64:128, :],
                                    bq_t[64:128, p:p + 1])
                            else:
                                nc.vector.tensor_scalar_add(
                                    kT_t[p][:, cs], ps[:], bk_t[:, p:p + 1])

                        units.append(u1)
                        units.append(u2)
                return units

            def v_units(p, sbs):
                """V projection for pair p: per s-block psum [128(s),128(vc)]."""
                units = []
                for sb in sbs:
                    def u(p=p, sb=sb):
                        ps = pfill.tile([128, 512], F32, tag="fill",
                                        name=f"psv{p}_{sb}")
                        for db in range(ND):
                            nc.tensor.matmul(
                                ps[:, 0:128],
                                lhsT=xt_t[db][:, sb * 128:(sb + 1) * 128],
                                rhs=wv_t[p][:, db * 128:(db + 1) * 128],
                                start=(db == 0), stop=(db == ND - 1),
                            )
                        vt = v_t[p][sb]
                        v3 = vt[:].rearrange("p (h e) -> p h e", e=HD + 1)
                        nc.vector.tensor_tensor(
                            v3[:, :, 0:HD],
                            ps[:, 0:128].rearrange("p (h e) -> p h e", e=HD),
                            bv_t[:, p * 128:(p + 1) * 128].rearrange(
                                "p (h e) -> p h e", e=HD),
                            op=ADD,
                        )
                        nc.vector.tensor_copy(
                            v3[:, :, HD:HD + 1],
                            ones_t[:].rearrange("p (h e) -> p h e", e=1),
                        )
                    units.append(u)
                return units

            def outproj_units(sb_list):
                units = []
                for sb in sb_list:
                    for jc in range(D // 512):
                        def u(sb=sb, jc=jc):
                            ps = pfill.tile([128, 512], F32, tag="fill",
                                            name=f"po{sb}_{jc}")
                            for db in range(NPAIR):
                                nc.tensor.matmul(
                                    ps[:],
                                    lhsT=aT_t[db][:, sb * 128:(sb + 1) * 128],
                                    rhs=wo_t[db][:, jc * 512:(jc + 1) * 512],
                                    start=(db == 0), stop=(db == NPAIR - 1),
                                )
                            ot = post.tile([128, 512], F32, tag="ostage",
                                           name=f"ot{sb}_{jc}")
                            nc.vector.tensor_copy(ot[:], ps[:])
                            nc.sync.dma_start(
                                out[sb * 128:(sb + 1) * 128,
                                    jc * 512:(jc + 1) * 512],
                                ot[:],
                            )
                        units.append(u)
                return units

            def make_popper(units, n_slots, skip_first=0):
                """Evenly pace `units` over `n_slots` popper() calls."""
                state = {"credit": 0.0, "slot": 0}
                rate = len(units) / max(n_slots - skip_first, 1)

                def popper():
                    state["slot"] += 1
                    if state["slot"] <= skip_first:
                        return
                    state["credit"] += rate
                    while units and state["credit"] >= 1.0:
                        state["credit"] -= 1.0
                        units.pop(0)()
                return popper, units

            # ================= attention =================
            def chunk_cols(lo):
                chunks = []
                c = lo
                while c < QH:
                    c1 = min((c // 512 + 1) * 512, QH)
                    chunks.append((c, c1))
                    c = c1
                return chunks

            pending_fin = []

            def emit_normalize(h, qh, at, c0, c1, defer=True):
                hb, hr = h // 2, (h % 2) * 64
                q0 = qh * QH
                w = c1 - c0
                au = pau.tile([65, w], F32, tag="au", name=f"au{h}_{qh}_{c0}")
                nc.scalar.copy(au[:], at[0:65, c0:c1])
                dn = prc.tile([1, w], F32, tag="dn", name=f"dn{h}_{qh}_{c0}")
                nc.vector.tensor_copy(dn[:], au[64:65, :])
                rc = prc.tile([1, w], F32, tag="rc", name=f"rc{h}_{qh}_{c0}")
                nc.vector.reciprocal_approx_fast(rc[:], dn[:])
                bc = pbc.tile([64, w], F32, tag="bc", name=f"bc{h}_{qh}_{c0}")
                nc.gpsimd.partition_broadcast(bc[:], rc[:])

                def fin():
                    nc.vector.tensor_tensor(
                        aT_t[hb][hr:hr + 64, q0 + c0:q0 + c1],
                        au[0:64, :],
                        bc[:],
                        op=MULT,
                    )
                # defer the multiply into the next attention loop so the
                # bcast wait never head-blocks the DVE queue (masks behind
                # it gate the PE); flushed before any aT consumer emits.
                if defer:
                    pending_fin.append(fin)
                else:
                    fin()

            def attn_head_qh(h, qh, popper, split_at=None):
                hb, hr = h // 2, (h % 2) * 64
                vcol = (h % 2) * (HD + 1)
                q0 = qh * QH
                at = patps.tile([65, QH], F32, tag="at", name=f"at{h}_{qh}")
                nkb = (q0 + QH) // 128
                kb_split = ((q0 + split_at - 1) // 128 if split_at is not None
                            else None)

                def scores(kb):
                    k0 = kb * 128
                    lo = max(k0 - q0, 0)
                    sc = pscps.tile([128, QH], F32, tag="sc",
                                    name=f"sc{h}_{qh}_{kb}")
                    for (c0, c1) in chunk_cols(lo):
                        nc.tensor.matmul(
                            sc[:, c0:c1],
                            lhsT=kT_t[hb][:, k0:k0 + 128],
                            rhs=qz_t[h][:, q0 + c0:q0 + c1],
                            start=True, stop=True,
                        )
                    return sc

                def exp_pv(kb, sc):
                    k0 = kb * 128
                    lo = max(k0 - q0, 0)
                    et = pexp.tile([128, QH], DT, tag="exp",
                                   name=f"et{h}_{qh}_{kb}")
                    nc.scalar.activation(et[:, lo:QH], sc[:, lo:QH],
                                         EXP, scale=1.0 / np.sqrt(HD))
                    if k0 >= q0:
                        nc.vector.tensor_mul(et[:, lo:lo + 128],
                                             et[:, lo:lo + 128],
                                             mask_t[:])
                    for (c0, c1) in chunk_cols(lo):
                        nc.tensor.matmul(
                            at[0:65, c0:c1],
                            lhsT=v_t[hb][kb][:, vcol:vcol + HD + 1],
                            rhs=et[:, c0:c1],
                            start=(kb == 0),
                            stop=(kb == (q0 + c1 - 1) // 128),
                        )

                # software pipeline: scores one kb ahead of exp+pv; filler
                # units run between scores and pv so the PE queue never
                # head-blocks on ACT's exp.
                prev = scores(0)
                for kb in range(1, nkb):
                    cur = scores(kb)
                    popper()
                    exp_pv(kb - 1, prev)
                    if kb == 3:
                        while pending_fin:
                            pending_fin.pop(0)()
                    if kb_split is not None and kb - 1 == kb_split:
                        # first column chunk fully accumulated: normalize it
                        # now so consumers (output projection) start early.
                        emit_normalize(h, qh, at, 0, split_at, defer=False)
                    prev = cur
                popper()
                exp_pv(nkb - 1, prev)
                if kb_split is not None:
                    emit_normalize(h, qh, at, split_at, QH, defer=False)
                else:
                    emit_normalize(h, qh, at, 0, QH)
                popper()

            # ================= schedule =================
            def run_pair(p, units):
                n_slots = 2 * sum((qh * QH + QH) // 128
                                  for qh in range(S // QH))
                popper, _ = make_popper(units, n_slots)
                for h in (2 * p, 2 * p + 1):
                    for qh in range(S // QH):
                        attn_head_qh(h, qh, popper)
                while units:
                    units.pop(0)()

            # minimal slice of pair-0 QKV up front: q/k s-cols 0..1024 and
            # v s-blocks 0..7 are all head 0 qh=0 needs, and they only read
            # the first x column-half, which lands early.
            for u in qk_units(0, [0, 1]) + v_units(0, range(8)):
                u()
            attn_head_qh(0, 0, lambda: None)
            for u in qk_units(0, [2, 3]):
                u()
            rest0 = (v_units(0, range(8, NS)) + qk_units(1, [0, 1])
                     + v_units(1, range(8)) + qk_units(1, [2, 3])
                     + v_units(1, range(8, NS)))
            popper, _ = make_popper(rest0, 40)
            attn_head_qh(0, 1, popper)
            attn_head_qh(1, 0, popper)
            attn_head_qh(1, 1, popper)
            while rest0:
                rest0.pop(0)()

            # pairs 1..2: attention overlapped with next pair's QKV
            run_pair(1, qk_units(2, range(NSC)) + v_units(2, range(NS)))
            run_pair(2, qk_units(3, [0, 1]) + v_units(3, range(8)))

            # pair 3: finish own QKV during qh=0; output-project the first
            # s-half during qh=1 (gated so aT of both qh=0 heads lands);
            # split-normalize the qh=1 heads so sb8-11 can run in the tail
            # of h7/qh=1 instead of serializing after the last normalize.
            p = NPAIR - 1
            u_a = qk_units(p, [2, 3])
            popper, _ = make_popper(u_a, 8)
            attn_head_qh(2 * p, 0, popper)
            u_b = v_units(p, range(8, NS))
            popper, _ = make_popper(u_b, 8)
            attn_head_qh(2 * p + 1, 0, popper)

            while pending_fin:
                pending_fin.pop(0)()
            u_c = outproj_units(range(NS // 2))
            popper, _ = make_popper(u_c, 28, skip_first=2)
            attn_head_qh(2 * p, 1, popper, split_at=512)
            u_d = outproj_units(range(NS // 2, NS // 2 + 4))
            state = {"slot": 0}

            def popper_h7():
                state["slot"] += 1
                if u_c:
                    u_c.pop(0)()
                elif state["slot"] >= 14 and u_d:
                    u_d.pop(0)()
                    if u_d:
                        u_d.pop(0)()
            attn_head_qh(2 * p + 1, 1, popper_h7, split_at=512)
            for u in u_c + u_d:
                u()

            # tail: last quarter of the output projection
            for u in outproj_units(range(NS // 2 + 4, NS)):
                u()

    nc.compile()
    return nc


def _get_nc(dt_mode):
    if dt_mode not in _CACHE:
        _CACHE[dt_mode] = _build(dt_mode)
    return _CACHE[dt_mode]


def _pair_major(w):
    # [D, DH] -> [NPAIR*128, ND*128]: row p*128+part, col db*128+c
    #   = w[db*128+part, p*128+c]
    return np.ascontiguousarray(
        w.reshape(8, 128, NPAIR, 128).transpose(2, 1, 0, 3).reshape(
            NPAIR * 128, 1024))


def make_in_maps(x, Wq_w, Wq_b, Wk_w, Wk_b, Wv_w, Wv_b, Wo_w, Wo_b, np_dt):
    in_maps = []
    for core in range(N_CORES):
        b, half = core // 2, core % 2
        sl = slice(half * DH, (half + 1) * DH)
        in_maps.append({
            "xT": np.ascontiguousarray(x[b].T).astype(np_dt),
            "wq": _pair_major(Wq_w[:, sl]).astype(np_dt),
            "wk": _pair_major(Wk_w[:, sl]).astype(np_dt),
            "wv": _pair_major(Wv_w[:, sl]).astype(np_dt),
            "wo": np.ascontiguousarray(Wo_w[sl, :]).astype(np_dt),
            "bq": np.ascontiguousarray(Wq_b[sl].reshape(-1, 128).T),
            "bk": np.ascontiguousarray(Wk_b[sl].reshape(-1, 128).T),
            "bv": np.broadcast_to(Wv_b[sl], (128, DH)).copy(),
        })
    return in_maps


def kernel(x, Wq_w, Wq_b, Wk_w, Wk_b, Wv_w, Wv_b, Wo_w, Wo_b):
    from concourse.bass_utils import run_bass_kernel_spmd

    np_dt = ml_dtypes.bfloat16 if DT_MODE == "bf16" else np.float32

    args = [np.asarray(a, np.float32) for a in
            (x, Wq_w, Wq_b, Wk_w, Wk_b, Wv_w, Wv_b, Wo_w, Wo_b)]
    x, Wq_w, Wq_b, Wk_w, Wk_b, Wv_w, Wv_b, Wo_w, Wo_b = args

    nc = _get_nc(DT_MODE)
    in_maps = make_in_maps(x, Wq_w, Wq_b, Wk_w, Wk_b, Wv_w, Wv_b, Wo_w, Wo_b,
                           np_dt)
    res = run_bass_kernel_spmd(nc, in_maps, list(range(N_CORES)))

    out = np.empty((B, S, D), np.float32)
    for b in range(B):
        out[b] = res.results[2 * b]["out"] + res.results[2 * b + 1]["out"] + Wo_b
    return out
